# revision 3
# baseline (speedup 1.0000x reference)
"""Trainium2 Bass kernel for nn_LstmModel (TF LSTM, T=256, F=64, H=32,
dense(1)+ELU head), data-parallel over 8 NeuronCores.

v5 (over v4): wave-major PSUM and sigmoid-output layouts so the two
waves' access patterns occupy DISJOINT byte ranges. v4's bank-major
sigma read [128,(4banks,256)] had a bounding box overlapping the other
wave's H writes, so the tile dependency tracker serialized the waves
(the period was both chains concatenated). Wave-major decouples them.
PSUM start=True discipline: the first matmul touching each 2 KiB
PSUM zero-region (per partition range) carries start=True -- the
I-gate matmuls are emitted per-wave (start=True) to mark the IJ
regions, the F-bias matmuls are per-wave (start=True) marking the OF
regions; everything else accumulates with start=False (bytes still
pending-zero are zeroed on first write). tmp1 back on DVE (GPSIMD
tensor_tensor measured slower, 829 ns vs 426).

Per-core layout (B_loc = 2048 = 4 chunks x 512):
  x host layout  [128=(chunk%2)*64+f, t, pair, 512b] fp16 (pre-transposed
                 and cast on host; no PE transposes on device)
  PSUM gate tile [128, 2, 4, 256] fp32 (wave, bank, col), banks =
                 [I, J, O, F], each bank chunk-packed rows; the (wave,
                 I+J) and (wave, O+F) pairs are the 2 KiB zero-regions
  c, h, tmp      [128 = chunk*32+u, 512b] fp16 chunk-packed
  j pre-scaled 2x everywhere: tanh(j) = 2 sig(2j) - 1
  gates X-pass: 8 matmuls/step (4 banks x 2 stacked-chunk-pairs, M=64)
  gates H-pass: 4 matmuls/wave (4-block-diag lhsT [128,128], M=128)
  f-gate +1 bias via K=1 ones-row matmul into the F bank
  2 waves (256-col halves) pipeline the serial recurrence chain
"""

import sys

import numpy as np

sys.path.insert(0, "/opt/trn_rl_repo")

B_FULL = 16384
T = 256
F = 64
H = 32
FORGET_BIAS = 1.0
N_CORES = 8
B_LOC = B_FULL // N_CORES          # 2048
N_CHUNK = 4
CB = B_LOC // N_CHUNK              # 512
NW = 2                             # waves
WB = CB // NW                      # 256
TB = 16                            # time steps per x DMA block

GATE_COLS = {"i": slice(0, 32), "j": slice(32, 64),
             "f": slice(64, 96), "o": slice(96, 128)}
BANK_GATES = ["i", "j", "o", "f"]  # PSUM bank order [I, J, O, F]

_CACHE = {}


def _build_kernel(t_steps, tb, bias_banks, bd_val):
    import concourse.tile as tile
    from concourse import bacc, mybir

    f32 = mybir.dt.float32
    f16 = mybir.dt.float16
    AF = mybir.ActivationFunctionType
    OP = mybir.AluOpType

    nc = bacc.Bacc(None, target_bir_lowering=False, debug=False)

    with tile.TileContext(nc) as tc:
        with tc.tile_pool(name="dram", bufs=1, space="DRAM") as dram:
            x_in = dram.tile([128, t_steps, 2, CB], f16,
                             kind="ExternalInput", name="x_in", uniquify=False)
            wxg_in = dram.tile([128, 4, 64], f16, kind="ExternalInput",
                               name="wxg_in", uniquify=False)
            whg_in = dram.tile([128, 4, 128], f16, kind="ExternalInput",
                               name="whg_in", uniquify=False)
            bias_in = dram.tile([1, 4, 128], f16, kind="ExternalInput",
                                name="bias_in", uniquify=False)
            wdbd_in = dram.tile([128, 4], f16, kind="ExternalInput",
                                name="wdbd_in", uniquify=False)
            out_ext = dram.tile([4, CB], f32, kind="ExternalOutput",
                                name="out_ext", uniquify=False)

            from contextlib import ExitStack
            stk = ExitStack()
            const = stk.enter_context(tc.tile_pool(name="const", bufs=1))
            wxg = const.tile([128, 4, 64], f16)
            whg = const.tile([128, 4, 128], f16)
            bias_l = const.tile([1, 4, 128], f16)
            wdbd = const.tile([128, 4], f16)
            ones = const.tile([1, CB], f16)
            nc.sync.dma_start(out=wxg[:], in_=wxg_in[:])
            nc.sync.dma_start(out=whg[:], in_=whg_in[:])
            nc.sync.dma_start(out=bias_l[:], in_=bias_in[:])
            nc.sync.dma_start(out=wdbd[:], in_=wdbd_in[:])
            nc.vector.memset(ones[:], 1.0)

            state = stk.enter_context(tc.tile_pool(name="state", bufs=1))
            c_st = state.tile([128, CB], f16)
            h_st = state.tile([128, CB], f16)
            sg = state.tile([128, NW, 4, WB], f16)
            tmp1 = state.tile([128, CB], f16)
            tmp2 = state.tile([128, CB], f16)
            tanh_c = state.tile([128, CB], f16)
            nc.vector.memset(c_st[:], 0.0)

            psum = stk.enter_context(
                tc.tile_pool(name="psum", bufs=1, space="PSUM"))
            g_tiles = [[psum.tile([128, 4, WB], f32, name=f"g{s}w{w}")
                        for w in range(NW)] for s in range(2)]

            xpool = stk.enter_context(tc.tile_pool(name="xpool", bufs=2))
            xblks = []
            for blk in range(t_steps // tb):
                xblk = xpool.tile([128, tb, 2, CB], f16, tag="xblk")
                nc.sync.dma_start(out=xblk[:],
                                  in_=x_in[:, blk * tb:(blk + 1) * tb])
                xblks.append(xblk)

            def g_of(t):
                return g_tiles[t % 2]

            def emit_gates_x(t, w):
                # Per (step, wave) tile [128, 4, WB]: zero-regions are the
                # I+J and O+F bank pairs. First toucher carries start=True:
                # I matmuls (per pr, marking IJ for their partitions) and
                # the F bias row (marking OF for all partitions). The rest
                # accumulate (pending-zero bytes are zeroed on write).
                g_w = g_of(t)[w]
                stop_x = (t == 0)
                xs_t = xblks[t // tb]
                wcols = slice(WB * w, WB * w + WB)
                for pr in range(2):
                    nc.tensor.matmul(
                        g_w[64 * pr:64 * pr + 64, 0, :], wxg[:, 0, :],
                        xs_t[:, t % tb, pr, wcols],
                        start=True, stop=stop_x,
                        tile_position=(0, 64 * pr), skip_group_check=True)
                nc.tensor.matmul(
                    g_w[:, 3, :], bias_l[:, 3, :], ones[:, 0:WB],
                    start=True, stop=False,
                    tile_position=(0, 0), skip_group_check=True)
                for g in (1, 2, 3):
                    for pr in range(2):
                        nc.tensor.matmul(
                            g_w[64 * pr:64 * pr + 64, g, :], wxg[:, g, :],
                            xs_t[:, t % tb, pr, wcols],
                            start=False, stop=stop_x,
                            tile_position=(0, 64 * pr),
                            skip_group_check=True)

            def emit_h_pass(t, w, hw, hc):
                # half-wave H update: 4 matmuls of N=WB/2 into the
                # half-columns hc of each gate bank
                g_w = g_of(t)[w]
                cols = slice((WB // 2) * hw, (WB // 2) * (hw + 1))
                for g in range(4):
                    nc.tensor.matmul(
                        g_w[:, g, cols], whg[:, g, :], h_st[:, hc],
                        start=False, stop=True,
                        tile_position=(0, 0), skip_group_check=True)

            for w in range(NW):
                emit_gates_x(0, w)
            for w in range(NW):
                emit_gates_x(1, w)

            wslices = [slice(w * WB, (w + 1) * WB) for w in range(NW)]

            for t in range(t_steps):
                g_t = g_of(t)

                # ACT: sigmoid per wave (contiguous [128, 4*WB])
                for w in range(NW):
                    nc.scalar.activation(sg[:, w, :, :], g_t[w][:, :, :],
                                         AF.Sigmoid)

                # chain per wave: tmp2(DVE) || tmp1(Pool), cupd, tanh
                for w, wc in enumerate(wslices):
                    nc.vector.scalar_tensor_tensor(
                        tmp2[:, wc], sg[:, w, 1, :], 0.5, sg[:, w, 0, :],
                        OP.subtract, OP.mult)
                    nc.vector.scalar_tensor_tensor(
                        tmp1[:, wc], c_st[:, wc], 1.0, sg[:, w, 3, :],
                        OP.mult, OP.mult)
                    nc.vector.scalar_tensor_tensor(
                        c_st[:, wc], tmp2[:, wc], 2.0, tmp1[:, wc],
                        OP.mult, OP.add)
                    nc.scalar.activation(tanh_c[:, wc], c_st[:, wc], AF.Tanh)

                # h then immediately its H(t+1) matmuls (chain-critical);
                # done in half-wave pieces so the first H quartet starts
                # as soon as the first half of h lands
                HW = WB // 2
                for w, wc in enumerate(wslices):
                    for hw in range(2):
                        hc = slice(wc.start + HW * hw,
                                   wc.start + HW * (hw + 1))
                        nc.vector.scalar_tensor_tensor(
                            h_st[:, hc], tanh_c[:, hc], 1.0,
                            sg[:, w, 2, HW * hw:HW * (hw + 1)],
                            OP.mult, OP.mult)
                        if t + 1 < t_steps:
                            emit_h_pass(t + 1, w, hw, hc)

                # prefetch x gates two steps ahead (keeps the PE fed; WAR
                # against this step's sigma which was emitted above)
                if t + 2 < t_steps:
                    for w in range(NW):
                        emit_gates_x(t + 2, w)

            # ---- dense head + ELU ----
            ybd = state.tile([4, CB], f32)
            m0 = state.tile([4, CB], f32)
            ex = state.tile([4, CB], f32)
            elu = state.tile([4, CB], f32)
            for w, wc in enumerate(wslices):
                nc.tensor.matmul(g_tiles[0][w][0:4, 0, :], wdbd[:],
                                 h_st[:, wc], start=True, stop=True,
                                 tile_position=(0, 0), skip_group_check=True)
                nc.vector.tensor_scalar_add(ybd[:, wc],
                                            g_tiles[0][w][0:4, 0, :],
                                            float(bd_val))
            nc.vector.tensor_scalar_min(m0[:], ybd[:], 0.0)
            nc.scalar.activation(ex[:], m0[:], AF.Exp)
            nc.vector.scalar_tensor_tensor(
                elu[:], ex[:], 1.0, ybd[:], OP.subtract, OP.max)
            nc.sync.dma_start(out=out_ext[:], in_=elu[:])
            stk.close()

    nc.compile()
    return nc


def _prep_weights(W_lstm, b_lstm, W_dense):
    Wx = W_lstm[:F, :].astype(np.float32)   # [64, 128]
    Wh = W_lstm[F:, :].astype(np.float32)   # [32, 128]
    b = b_lstm.astype(np.float32)
    scale = {"i": 1.0, "j": 2.0, "o": 1.0, "f": 1.0}
    badd = {"i": 0.0, "j": 0.0, "o": 0.0, "f": FORGET_BIAS}

    wxg = np.zeros((128, 4, 64), np.float32)
    whg = np.zeros((128, 4, 128), np.float32)
    bias_l = np.zeros((1, 4, 128), np.float32)
    for gi, g in enumerate(BANK_GATES):
        s = scale[g]
        for cc in range(2):
            wxg[64 * cc:64 * cc + 64, gi, 32 * cc:32 * cc + 32] = \
                s * Wx[:, GATE_COLS[g]]
        for ch in range(4):
            whg[32 * ch:32 * ch + 32, gi, 32 * ch:32 * ch + 32] = \
                s * Wh[:, GATE_COLS[g]]
        bias_l[0, gi, :] = np.tile(s * b[GATE_COLS[g]] + badd[g], 4)

    wdbd = np.zeros((128, 4), np.float32)
    for k in range(4):
        wdbd[32 * k:32 * k + 32, k] = W_dense[:, 0]

    bias_banks = tuple(bool(np.any(bias_l[0, gi] != 0.0)) for gi in range(4))
    return (wxg.astype(np.float16), whg.astype(np.float16),
            bias_l.astype(np.float16), wdbd.astype(np.float16), bias_banks)


def _prep_x(x):
    # full x [B_FULL, T*F] fp32 -> per-core [128, T, 2, CB] fp16 where
    # partition p = 64*(chunk%2) + f, pair = chunk//2, col = batch%CB
    xs = []
    for c in range(N_CORES):
        xc = x[c * B_LOC:(c + 1) * B_LOC].reshape(2, 2, CB, T, F)
        a = np.ascontiguousarray(
            xc.transpose(1, 4, 3, 0, 2).reshape(128, T, 2, CB)
            .astype(np.float16))
        xs.append(a)
    return xs


def kernel(x, W_lstm, b_lstm, W_dense, b_dense):
    from concourse.bass_utils import run_bass_kernel_spmd

    x = np.asarray(x, np.float32)
    wxg, whg, bias_l, wdbd, bias_banks = _prep_weights(
        np.asarray(W_lstm, np.float32), np.asarray(b_lstm, np.float32),
        np.asarray(W_dense, np.float32))
    bd_val = float(np.asarray(b_dense).reshape(-1)[0])

    key = (T, TB, bias_banks, bd_val)
    if key not in _CACHE:
        _CACHE[key] = _build_kernel(T, TB, bias_banks, bd_val)
    nc = _CACHE[key]

    xs = _prep_x(x)
    in_maps = [{"x_in": xs[c], "wxg_in": wxg, "whg_in": whg,
                "bias_in": bias_l, "wdbd_in": wdbd}
               for c in range(N_CORES)]

    res = run_bass_kernel_spmd(nc, in_maps, core_ids=list(range(N_CORES)))
    global LAST_EXEC_NS
    LAST_EXEC_NS = res.exec_time_ns
    outs = [r["out_ext"].reshape(-1) for r in res.results]
    return np.concatenate(outs).astype(np.float32)


LAST_EXEC_NS = None
